# revision 1
# baseline (speedup 1.0000x reference)
"""FAPE loss kernel for Trainium2 (8 NeuronCores, Bass/Tile).

Math
----
The reference computes, for frames i and residue-atoms (l, j):

    local[i, lj, d] = sum_c coords[lj, c] * R[i, d, c] - off[i, d]
    d2[i, lj]       = sum_d (pred_local - true_local)^2
    loss            = sum_{i,lj} m[i] * m[l] * min(sqrt(d2 + eps), 10) / ((sum m)^2 * 3 + eps) / 10

The delta is linear in the 7-vector u'[lj] = [pred_coords(3), true_coords(3), 1]:
    delta_d[i, lj] = dot(u'[lj], w_d[i]),  w_d[i] = [pR[i,d,:], -tR[i,d,:], -(offp-offt)[i,d]]
so d2 is a quadratic form
    d2[i, lj] = sum_{a<=b} mult_ab * u'_a u'_b * Q[i,(a,b)],  Q[i] = sum_d w_d w_d^T

Host (O(L) work): builds P[28, 6144] = pairwise products of u' (residue mask folded
in as zeroed columns, so masked entries give d2=0 -> dist 0) and Qv[i, 28], then
splits both into bf16 hi/lo halves and stacks the three cross terms
(Qh.Ph + Qh.Pl + Ql.Ph) along the contraction axis: the PE's matmul cost is
N-cycles regardless of K, so one K=84 bf16 matmul gives fp32-grade d2
(validated: end-to-end loss error ~3e-8) at ~10x the speed of a native fp32
matmul (which runs as two half-rate passes).

Device (O(L^2) work): d2 = A^T.T @ B as K=84 bf16 matmuls (N=512 each, two
matmuls fill one 2-bank PSUM group tile, four tiles in flight), then per
1024-wide group:
clamp to [0, 100] on the vector engine (min(sqrt(d2), 10) == sqrt(min(d2, 100));
max(.,0) guards bf16-split cancellation), sqrt + free-axis sum fused on the
scalar engine.  Each of the 8 cores handles 256 frames and returns 6 per-group
partition sums; the host folds them into per-frame sums, applies the frame mask
and final normalization.  eps inside the sqrt is dropped: its contribution is
O(1e-9) relative on this data.

Schedule: input arrives as six DMAs (Q + four chunk-aligned block0 pieces,
then blocks 1 and 2) alternating the two HW-DGE rings, so compute on block0
overlaps the remaining transfer; standalone bf16 LDWEIGHTS instructions absorb
the later blocks' DMA waits on the PE.  Groups run block-major; the steady
state is paced by the vector engine (PSUM-source fp32 tensor_scalar is 1x,
~1.2us per 1024-wide group).  PSUM RAW tracking is per-tile, not per-region,
so smaller group tiles are what lets the clamp start right behind the matmuls.

Toolchain constraint: this walrus build allows ONE semaphore wait per
instruction.  Chunk-aligned DMAs, no-reuse SBUF pools, the LDWEIGHTS wait
absorbers, and the scalar-engine dummy-activation chain keep every compute
instruction at <=1 wait; remaining multi-wait instructions (the Tile exit
drain) are split onto single-wait no-ops by _split_multi_waits.  Tile's
entry/exit all-engine barriers run in sem-only form (no per-engine drains).
"""

import sys

import numpy as np

for _p in ("/opt/trn_rl_repo",):
    if _p not in sys.path:
        sys.path.insert(0, _p)

import ml_dtypes
import concourse.bass as bass
import concourse.tile as tile
from concourse import mybir
from concourse.bass_utils import run_bass_kernel_spmd

L = 2048
N_CORES = 8
FRAMES_PER_CORE = L // N_CORES  # 256
NLJ = L * 3  # 6144
K = 28         # 7*8/2 upper-triangle pairs
KS = 3 * K     # 84: three bf16 cross terms stacked on the contraction axis
N_CHUNK = 512
GROUP_CHUNKS = 2
GROUP_COLS = GROUP_CHUNKS * N_CHUNK  # 1024 = one 2-bank PSUM tile
BLOCK_COLS = 2048                    # DMA block; holds 2 lj-groups
N_BLOCKS = NLJ // BLOCK_COLS         # 3
GROUPS_PER_BLOCK_LJ = BLOCK_COLS // GROUP_COLS  # 2
F_TILES = FRAMES_PER_CORE // 128     # 2
N_GROUPS = N_BLOCKS * GROUPS_PER_BLOCK_LJ * F_TILES  # 12
CLAMP2 = 100.0  # CLAMP_DISTANCE ** 2

_PAIRS = [(a, b) for a in range(7) for b in range(a, 7)]


def _host_prep(pred_coords, true_coords, pred_rotation, pred_translation,
               true_rotation, true_translation, mask):
    """Return (B (84, 6144) bf16, A (L, 84) bf16): the stacked hi/lo splits of
    the quadratic-form factors.  All O(L) flops, float64."""
    pc = np.asarray(pred_coords, np.float64)
    tc = np.asarray(true_coords, np.float64)
    pR = np.asarray(pred_rotation, np.float64)
    pT = np.asarray(pred_translation, np.float64)
    tR = np.asarray(true_rotation, np.float64)
    tT = np.asarray(true_translation, np.float64)

    UT = np.concatenate([
        pc.reshape(L * 3, 3).T,
        tc.reshape(L * 3, 3).T,
        np.ones((1, L * 3)),
    ], axis=0)  # (7, 6144)

    offp = np.einsum('ic,idc->id', pT, pR)
    offt = np.einsum('ic,idc->id', tT, tR)
    W = np.concatenate([pR, -tR, -(offp - offt)[:, :, None]], axis=2)  # (L, 3, 7)
    Q = np.einsum('ida,idb->iab', W, W)  # (L, 7, 7)

    Qv = np.stack([Q[:, a, b] * (1.0 if a == b else 2.0) for (a, b) in _PAIRS],
                  axis=1).astype(np.float32)  # (L, 28)
    P = np.stack([UT[a] * UT[b] for (a, b) in _PAIRS], axis=0)  # (28, 6144)

    m_lj = np.repeat(np.asarray(mask, np.float64) != 0, 3)
    P32 = (P * m_lj[None, :]).astype(np.float32)

    def split(x):
        hi = x.astype(ml_dtypes.bfloat16)
        lo = (x - hi.astype(np.float32)).astype(ml_dtypes.bfloat16)
        return hi, lo

    Ph, Pl = split(P32)
    Qh, Ql = split(Qv)
    B = np.concatenate([Ph, Pl, Ph], axis=0)   # (84, 6144)
    A = np.concatenate([Qh, Qh, Ql], axis=1)   # (L, 84)
    return np.ascontiguousarray(B), np.ascontiguousarray(A)


def _split_multi_waits(nc):
    """The TPB instruction encodings used by this walrus build carry a single
    semaphore wait.  Tile can emit several waits on one instruction (notably
    the kernel-tail drain).  Split the extras onto same-engine no-ops placed
    immediately before the instruction — engine-order execution makes this
    semantically identical."""
    for bbw in nc.main_func.blocks:
        il = bbw.instructions
        out = []
        changed = False
        for ins in il:
            si = ins.sync_info
            if si is not None and len(si.on_wait) > 1:
                waits = list(si.on_wait)
                for idx, w in enumerate(waits[:-1]):
                    out.append(mybir.InstNoOp(
                        name=f"{ins.name}-waitsplit{idx}",
                        engine=ins.engine,
                        sync_info=mybir.SyncInfo(on_wait=[w], on_update=[]),
                    ))
                si.on_wait = [waits[-1]]
                changed = True
            out.append(ins)
        if changed:
            bbw.instructions = out


def _build_program(split_waits=True):
    f32 = mybir.dt.float32
    bf16 = mybir.dt.bfloat16
    # Tile's entry/exit all-engine barriers default to the drain+EVSEM
    # butterfly; the sem-only variant synchronizes the same points without
    # the drains (~0.7us saved, measured; correctness preserved since the
    # kernel-tail drain instruction is still emitted separately).
    _orig_aeb = bass.Bass.all_engine_barrier
    bass.Bass.all_engine_barrier = (
        lambda self, *, sem_only=False: _orig_aeb(self, sem_only=True))
    try:
        nc = _build_program_inner(f32, bf16, split_waits)
    finally:
        bass.Bass.all_engine_barrier = _orig_aeb
    return nc


def _build_program_inner(f32, bf16, split_waits):
    nc = bass.Bass()
    # Input layout: [Q (256) | lj block0 (2048) | block1 (2048) | block2 (2048)],
    # loaded by six DMAs (Q + four chunk-aligned block0 pieces, then blocks 1
    # and 2) so compute on block0 overlaps the remaining transfers.
    inp = nc.declare_dram_parameter("inp", [KS, FRAMES_PER_CORE + NLJ], bf16,
                                    isOutput=False)
    # Raw per-group accumulator; host folds the 12 columns into frame sums.
    fsums = nc.declare_dram_parameter("fsums", [128, N_GROUPS], f32,
                                      isOutput=True)
    Q0 = FRAMES_PER_CORE  # column where lj blocks start

    with tile.TileContext(nc) as tc:
        with tc.tile_pool(name="const", bufs=1) as const_pool, \
             tc.tile_pool(name="clamped", bufs=N_GROUPS) as clamped_pool, \
             tc.tile_pool(name="ps", bufs=4, space="PSUM") as ps:
            data = const_pool.tile([KS, FRAMES_PER_CORE + NLJ], bf16)
            # Block0 (+Q) arrives as four chunk-aligned DMAs so each of the
            # first four matmuls waits on exactly its own chunk and compute
            # starts ~2us after the first chunk lands; blocks 1 and 2 stream
            # in behind the compute.
            bounds = [0, Q0 + N_CHUNK, Q0 + 2 * N_CHUNK, Q0 + 3 * N_CHUNK,
                      Q0 + BLOCK_COLS, Q0 + 2 * BLOCK_COLS, Q0 + 3 * BLOCK_COLS]
            # Alternate the two HW-DGE rings (SP and ACT sequencers): DMA
            # issue costs ~0.7us on the issuing engine, so splitting the six
            # issues across two engines halves the serial issue latency.
            engines = [nc.sync, nc.scalar, nc.sync, nc.scalar, nc.sync, nc.scalar]
            for i in range(6):
                engines[i].dma_start(data[:, bounds[i]:bounds[i + 1]],
                                     inp[:, bounds[i]:bounds[i + 1]])

            acc = const_pool.tile([128, N_GROUPS], f32)

            # Scalar-engine constant + two dummy activations: the sqrt bias
            # const-AP and the engine's own-semaphore ticks would otherwise
            # put a second wait on the first real sqrt (walrus allows one).
            bias_t = const_pool.tile([128, 1], f32)
            scratch_t = const_pool.tile([128, 1], f32)
            nc.scalar.memzero(bias_t[:])
            nc.scalar.activation(bias_t[:], bias_t[:],
                                 mybir.ActivationFunctionType.Sqrt,
                                 bias=bias_t[:, 0:1])
            nc.scalar.activation(scratch_t[:], bias_t[:],
                                 mybir.ActivationFunctionType.Sqrt,
                                 bias=bias_t[:, 0:1])

            # Group order: block-major (so block-b compute overlaps the DMA
            # of block b+1), then frame tile, then lj half-block.
            # g = b*4 + f*2 + h; group columns = block b cols [h*1024,(h+1)*1024).
            for g in range(N_GROUPS):
                b = g // (F_TILES * GROUPS_PER_BLOCK_LJ)
                f = (g // GROUPS_PER_BLOCK_LJ) % F_TILES
                h = g % GROUPS_PER_BLOCK_LJ
                if f == 0 and h == 0 and b > 0:
                    # Standalone bf16 LDWEIGHTS as a pure wait-carrier: it
                    # absorbs block-b's DMA-queue wait on the PE so the real
                    # matmuls only ever wait on their PSUM-slot release
                    # (single-wait-per-instruction toolchain limit).
                    nc.tensor.ldweights(
                        data[:, Q0 + b * BLOCK_COLS:Q0 + b * BLOCK_COLS + 128])
                d2 = ps.tile([128, GROUP_COLS], f32, tag="d2")
                for c in range(GROUP_CHUNKS):
                    col = Q0 + b * BLOCK_COLS + h * GROUP_COLS + c * N_CHUNK
                    nc.tensor.matmul(
                        d2[:, c * N_CHUNK:(c + 1) * N_CHUNK],
                        data[:, f * 128:(f + 1) * 128],
                        data[:, col:col + N_CHUNK],
                        start=True, stop=True,
                    )
                clamped = clamped_pool.tile([128, GROUP_COLS], f32,
                                            tag="clamped")
                nc.vector.tensor_scalar(
                    out=clamped[:], in0=d2[:],
                    scalar1=0.0, scalar2=CLAMP2,
                    op0=mybir.AluOpType.max, op1=mybir.AluOpType.min,
                )
                nc.scalar.activation(
                    clamped[:], clamped[:],
                    mybir.ActivationFunctionType.Sqrt,
                    bias=bias_t[:, 0:1],
                    accum_out=acc[:, g:g + 1],
                )

            # Fresh HW-DGE lane: single data-ready wait.
            nc.sync.dma_start(fsums[:], acc[:])
    if split_waits:
        # Needed for the walrus compile; CoreSim can't model the raw no-ops.
        _split_multi_waits(nc)
    return nc


def kernel(pred_coords, true_coords, pred_rotation, pred_translation,
           true_rotation, true_translation, mask, **_run_kwargs):
    mask = np.asarray(mask)
    B, A = _host_prep(pred_coords, true_coords, pred_rotation,
                      pred_translation, true_rotation, true_translation, mask)

    in_maps = []
    for c in range(N_CORES):
        a_c = A[c * FRAMES_PER_CORE:(c + 1) * FRAMES_PER_CORE].T  # (84, 256)
        in_maps.append({"inp": np.ascontiguousarray(
            np.concatenate([a_c, B], axis=1))})  # (84, 6400)

    nc = _build_program()
    res = run_bass_kernel_spmd(nc, in_maps, list(range(N_CORES)),
                               **_run_kwargs)

    m_i = np.asarray(mask, np.float64)
    numer = 0.0
    for c in range(N_CORES):
        fs = np.asarray(res.results[c]["fsums"], np.float64)  # (128, 12)
        # acc column g = b*4 + f*2 + h; frame index = c*256 + f*128 + p
        g = fs.reshape(128, N_BLOCKS, F_TILES, GROUPS_PER_BLOCK_LJ)
        frame_sums = g.sum(axis=(1, 3)).T.reshape(-1)
        numer += float((m_i[c * FRAMES_PER_CORE:(c + 1) * FRAMES_PER_CORE]
                        * frame_sums).sum())

    denom = float(m_i.sum()) ** 2 * 3.0 + 1e-8
    out = np.float32(numer / denom / 10.0)
    if _run_kwargs:
        return out, res
    return out



# revision 3
# speedup vs baseline: 1.4783x; 1.4783x over previous
"""FAPE loss kernel for Trainium2 (8 NeuronCores, Bass/Tile) — v2.

Math
----
The reference computes, for frames i and residue-atoms (l, j):

    local[i, lj, d] = sum_c coords[lj, c] * R[i, d, c] - off[i, d]
    d2[i, lj]       = sum_d (pred_local - true_local)^2
    loss            = sum_{i,lj} m[i] * m[l] * min(sqrt(d2), 10) / ((sum m)^2 * 3 + eps) / 10

d2 is a quadratic form in the 7-vector u'[lj] = [pred_coords(3), true_coords(3), 1]:
    d2[i, lj] = sum_{a<=b} mult_ab * u'_a u'_b * Q[i,(a,b)]
so on device it is a K=84 bf16 matmul (three stacked hi/lo cross terms
Qh.Ph + Qh.Pl + Ql.Ph, fp32-grade: residual ~ |Ql||Pl| ~ 1e-3 absolute).

v2 changes vs v1 (31.5us):
1. MASK COMPACTION.  Both the frame axis i and the residue axis l are
   gated by the same mask; v1 computed all 2048x6144 elements and zeroed
   masked columns.  v2 keeps only valid frames/residues: the device
   processes min(nv, 1024) frames (one 128-frame tile per core) x 3*nv
   columns — ~4x less work for nv~1024.  Overflow frames (nv mod 1024,
   when small) are summed exactly on the host in numpy (O(nv) of the
   O(nv^2) total).
2. SCALAR-FIRST POST-PROCESSING.  v1: DVE clamp from PSUM (1x fp32,
   1.19us/KFD) -> ACT sqrt+accum (1.24us/KFD) — two ~1x passes.  v2: ACT
   reads PSUM directly, sqrt -> SBUF bf16 (the one unavoidable 1x pass,
   (172+FD)/1.2ns), then DVE does min(dist,10) + free-axis sum in ONE
   tensor_scalar with accum_out — bf16/SBUF/step-1 keeps all fast perf
   modes (4x_2P: (58+FD/4)/0.96ns).  Steady state is ACT-paced.
3. sqrt(d2 + b), b=4e-3, guards bf16-split cancellation (d2_split can be
   ~-1e-3 where true d2 ~ 0; sqrt(neg) = NaN).  Systematic effect on the
   loss ~ +b/(2*dist) per element ~ 5e-5 relative — far under the 2e-2
   gate (validated vs reference).

Schedule: input [A (tpc*128) | B (3*nv)] per core arrives as ~1-group
DMA pieces on the SP HW-DGE ring so group-g matmuls wait only on piece g;
the ACT table load + bias-const waits are absorbed at t=0 by a dummy
activation chain (bias tile memset by the idle DVE); a standalone
LDWEIGHTS absorbs piece-0's wait on the PE.  PSUM: one (128,1024) fp32
tile per column group, <=8 banks, no reuse for the common nv<=1024 case.

Toolchain constraint: this walrus build allows ONE semaphore wait per
instruction; remaining multi-wait instructions (the Tile exit drain) are
split onto single-wait no-ops by _split_multi_waits.  Tile's entry/exit
all-engine barriers run in sem-only form (no per-engine drains).
"""

import sys

import numpy as np

for _p in ("/opt/trn_rl_repo",):
    if _p not in sys.path:
        sys.path.insert(0, _p)

import ml_dtypes
import concourse.bass as bass
import concourse.tile as tile
from concourse import mybir
from concourse.bass_utils import run_bass_kernel_spmd

L = 2048
N_CORES = 8
K = 28         # 7*8/2 upper-triangle pairs
KS = 3 * K     # 84: three bf16 cross terms stacked on the contraction axis
CHUNK = 512    # matmul N (one PSUM bank)
GROUP = 1024   # cols per ACT/DVE instruction (2 chunks, one 2-bank PSUM tile)
CLAMP = 10.0
B_EPS = 4e-3   # bias under the sqrt; guards split-cancellation negatives

_PAIRS = [(a, b) for a in range(7) for b in range(a, 7)]
_DIAG_COLS = [k for k, (a, b) in enumerate(_PAIRS) if a == b]


def _split(x):
    hi = x.astype(ml_dtypes.bfloat16)
    lo = (x - hi.astype(np.float32)).astype(ml_dtypes.bfloat16)
    return hi, lo


def _host_prep(pred_coords, true_coords, pred_rotation, pred_translation,
               true_rotation, true_translation, idx):
    """Quadratic-form factors for the compacted (valid-only) problem.

    Returns (B (84, 3*nv) bf16, A (nv, 84) bf16, Qv (nv, 28) f64,
    P64 (28, 3*nv) f64).  All O(L) flops."""
    pc = np.asarray(pred_coords, np.float64)[idx]
    tc = np.asarray(true_coords, np.float64)[idx]
    pR = np.asarray(pred_rotation, np.float64)[idx]
    pT = np.asarray(pred_translation, np.float64)[idx]
    tR = np.asarray(true_rotation, np.float64)[idx]
    tT = np.asarray(true_translation, np.float64)[idx]
    nv = len(idx)

    UT = np.concatenate([
        pc.reshape(nv * 3, 3).T,
        tc.reshape(nv * 3, 3).T,
        np.ones((1, nv * 3)),
    ], axis=0)  # (7, 3*nv)

    offp = np.einsum('ic,idc->id', pT, pR)
    offt = np.einsum('ic,idc->id', tT, tR)
    W = np.concatenate([pR, -tR, -(offp - offt)[:, :, None]], axis=2)  # (nv, 3, 7)
    Q = np.einsum('ida,idb->iab', W, W)  # (nv, 7, 7)

    Qv = np.stack([Q[:, a, b] * (1.0 if a == b else 2.0) for (a, b) in _PAIRS],
                  axis=1)  # (nv, 28) f64
    P64 = np.stack([UT[a] * UT[b] for (a, b) in _PAIRS], axis=0)  # (28, 3*nv)

    Ph, Pl = _split(P64.astype(np.float32))
    Qh, Ql = _split(Qv.astype(np.float32))
    B = np.concatenate([Ph, Pl, Ph], axis=0)   # (84, 3*nv)
    A = np.concatenate([Qh, Qh, Ql], axis=1)   # (nv, 84)
    return np.ascontiguousarray(B), np.ascontiguousarray(A), Qv, P64


def _split_multi_waits(nc):
    """The TPB instruction encodings used by this walrus build carry a single
    semaphore wait.  Tile can emit several waits on one instruction (notably
    the kernel-tail drain).  Split the extras onto same-engine no-ops placed
    immediately before the instruction — engine-order execution makes this
    semantically identical."""
    for bbw in nc.main_func.blocks:
        il = bbw.instructions
        out = []
        changed = False
        for ins in il:
            si = ins.sync_info
            if si is not None and len(si.on_wait) > 1:
                waits = list(si.on_wait)
                for idx_, w in enumerate(waits[:-1]):
                    out.append(mybir.InstNoOp(
                        name=f"{ins.name}-waitsplit{idx_}",
                        engine=ins.engine,
                        sync_info=mybir.SyncInfo(on_wait=[w], on_update=[]),
                    ))
                si.on_wait = [waits[-1]]
                changed = True
            out.append(ins)
        if changed:
            bbw.instructions = out


def _build_program(tpc, widths, split_waits=True):
    """tpc: frame tiles per core (usually 1); widths: per-group column
    widths (each <= GROUP, even)."""
    _orig_aeb = bass.Bass.all_engine_barrier
    bass.Bass.all_engine_barrier = (
        lambda self, *, sem_only=False: _orig_aeb(self, sem_only=True))
    try:
        nc = _build_program_inner(tpc, widths)
    finally:
        bass.Bass.all_engine_barrier = _orig_aeb
    if split_waits:
        _split_multi_waits(nc)
    return nc


def _build_program_inner(tpc, widths):
    f32 = mybir.dt.float32
    bf16 = mybir.dt.bfloat16
    C = sum(widths)
    A_COLS = tpc * 128
    NGROUPS = tpc * len(widths)

    nc = bass.Bass()
    inp = nc.declare_dram_parameter("inp", [KS, A_COLS + C], bf16,
                                    isOutput=False)
    fsums = nc.declare_dram_parameter("fsums", [128, NGROUPS], f32,
                                      isOutput=True)

    # DMA pieces: piece 0 = all A tiles + group 0; then one piece per
    # remaining group, except tiny trailing groups ride with their
    # predecessor.  Group g's matmuls then wait on exactly piece g's sem.
    bounds = [0]
    off = A_COLS
    for gi, w in enumerate(widths):
        off += w
        if gi + 1 < len(widths) and widths[gi + 1] < CHUNK:
            continue  # merge the tiny trailing group into this piece
        bounds.append(off)
    if bounds[-1] != A_COLS + C:
        bounds.append(A_COLS + C)

    with tile.TileContext(nc) as tc:
        with tc.tile_pool(name="const", bufs=1) as const_pool, \
             tc.tile_pool(name="dist", bufs=NGROUPS) as dist_pool, \
             tc.tile_pool(name="ps", bufs=min(NGROUPS, 4), space="PSUM") as ps:
            data = const_pool.tile([KS, A_COLS + C], bf16)
            for i in range(len(bounds) - 1):
                nc.sync.dma_start(data[:, bounds[i]:bounds[i + 1]],
                                  inp[:, bounds[i]:bounds[i + 1]])

            acc = const_pool.tile([128, NGROUPS], f32)
            bias_t = const_pool.tile([128, 1], f32)
            scratch_t = const_pool.tile([128, 1], f32)
            # Idle DVE fills the sqrt-bias const; the dummy activation chain
            # on ACT absorbs the bias wait AND triggers the sqrt table load
            # (PWP, ~1.3us) at t~0, hidden under the input DMA.
            nc.vector.memset(bias_t[:], B_EPS)
            nc.scalar.activation(scratch_t[:], bias_t[:],
                                 mybir.ActivationFunctionType.Sqrt,
                                 bias=bias_t[:, 0:1])
            nc.scalar.activation(scratch_t[:], bias_t[:],
                                 mybir.ActivationFunctionType.Sqrt,
                                 bias=bias_t[:, 0:1])

            for t in range(tpc):
                col = A_COLS
                for gi, w in enumerate(widths):
                    g = t * len(widths) + gi
                    if gi == 0:
                        # Standalone LDWEIGHTS absorbs the DMA-queue wait on
                        # the PE so the real matmuls only wait on their PSUM
                        # slot (single-wait-per-instruction limit).
                        nc.tensor.ldweights(data[:, t * 128:t * 128 + 128])
                    d2 = ps.tile([128, w], f32, tag="d2")
                    for c0 in range(0, w, CHUNK):
                        cw = min(CHUNK, w - c0)
                        nc.tensor.matmul(
                            d2[:, c0:c0 + cw],
                            data[:, t * 128:(t + 1) * 128],
                            data[:, col + c0:col + c0 + cw],
                            start=True, stop=True,
                        )
                    dist = dist_pool.tile([128, w], bf16, tag="dist")
                    nc.scalar.activation(
                        dist[:], d2[:],
                        mybir.ActivationFunctionType.Sqrt,
                        bias=bias_t[:, 0:1],
                    )
                    clamp = dist_pool.tile([128, w], bf16, tag="clamp")
                    # With accum_out, op1 is the reduction op (walrus
                    # TensorScalarPtrReduce form): out = min(in0, 10),
                    # accum_out = sum(out).
                    nc.vector.tensor_scalar(
                        out=clamp[:], in0=dist[:],
                        scalar1=CLAMP, scalar2=None,
                        op0=mybir.AluOpType.min,
                        op1=mybir.AluOpType.add,
                        accum_out=acc[:, g:g + 1],
                    )
                    col += w

            nc.sync.dma_start(fsums[:], acc[:])
    return nc


_PROGRAM_CACHE = {}


def _get_program(tpc, widths):
    key = (tpc, tuple(widths))
    if key not in _PROGRAM_CACHE:
        _PROGRAM_CACHE[key] = _build_program(tpc, widths)
    return _PROGRAM_CACHE[key]


def kernel(pred_coords, true_coords, pred_rotation, pred_translation,
           true_rotation, true_translation, mask, **_run_kwargs):
    mask = np.asarray(mask)
    idx = np.nonzero(mask != 0)[0]
    nv = len(idx)
    if nv == 0:
        out = np.float32(0.0)
        return (out, None) if _run_kwargs else out

    B, A, Qv, P64 = _host_prep(pred_coords, true_coords, pred_rotation,
                               pred_translation, true_rotation,
                               true_translation, idx)
    C = 3 * nv

    # Frame split: device takes n_dev = tpc*1024 (one or more full 128-frame
    # tiles per core); a small overflow is summed exactly on the host.
    tpc = max(1, int(round(nv / 1024)))
    n_dev = min(nv, tpc * 1024)
    n_off = nv - n_dev

    # Column groups: full GROUPs then an (even) remainder.
    C_pad = C + (C & 1)
    widths = [GROUP] * (C_pad // GROUP)
    if C_pad % GROUP:
        widths.append(C_pad % GROUP)
    NGROUPS = tpc * len(widths)

    A_pad = np.zeros((tpc * 1024, KS), A.dtype)
    A_pad[:n_dev] = A[:n_dev]
    B_pad = np.zeros((KS, C_pad), B.dtype)
    B_pad[:, :C] = B

    in_maps = []
    for c in range(N_CORES):
        a_c = A_pad[c * tpc * 128:(c + 1) * tpc * 128].T  # (84, tpc*128)
        in_maps.append({"inp": np.ascontiguousarray(
            np.concatenate([a_c, B_pad], axis=1))})

    nc = _get_program(tpc, widths)
    res = run_bass_kernel_spmd(nc, in_maps, list(range(N_CORES)),
                               **_run_kwargs)

    numer = 0.0
    sqrt_b = float(np.sqrt(B_EPS))  # pad-column contribution (approx; tiny)
    n_pad_cols = C_pad - C
    for c in range(N_CORES):
        fs = np.asarray(res.results[c]["fsums"], np.float64)  # (128, NGROUPS)
        for t in range(tpc):
            f0 = (c * tpc + t) * 128
            cnt = min(128, max(0, n_dev - f0))
            if cnt == 0:
                continue
            s = fs[:cnt, t * len(widths):(t + 1) * len(widths)].sum()
            s -= cnt * n_pad_cols * sqrt_b
            numer += s

    if n_off:
        # Exact host sum for the overflow frames (O(n_off * 3nv) elements).
        d2 = Qv[n_dev:nv] @ P64  # (n_off, 3*nv)
        numer += np.minimum(np.sqrt(np.maximum(d2, 0.0)), CLAMP).sum()

    denom = float(nv) ** 2 * 3.0 + 1e-8
    out = np.float32(numer / denom / 10.0)
    if _run_kwargs:
        return out, res
    return out


# revision 6
# speedup vs baseline: 1.5814x; 1.0698x over previous
"""FAPE loss kernel for Trainium2 (8 NeuronCores, Bass/Tile) — v2.

Math
----
The reference computes, for frames i and residue-atoms (l, j):

    local[i, lj, d] = sum_c coords[lj, c] * R[i, d, c] - off[i, d]
    d2[i, lj]       = sum_d (pred_local - true_local)^2
    loss            = sum_{i,lj} m[i] * m[l] * min(sqrt(d2), 10) / ((sum m)^2 * 3 + eps) / 10

d2 is a quadratic form in the 7-vector u'[lj] = [pred_coords(3), true_coords(3), 1]:
    d2[i, lj] = sum_{a<=b} mult_ab * u'_a u'_b * Q[i,(a,b)]
so on device it is a K=84 bf16 matmul (three stacked hi/lo cross terms
Qh.Ph + Qh.Pl + Ql.Ph, fp32-grade: residual ~ |Ql||Pl| ~ 1e-3 absolute).

v2 changes vs v1 (31.5us):
1. MASK COMPACTION.  Both the frame axis i and the residue axis l are
   gated by the same mask; v1 computed all 2048x6144 elements and zeroed
   masked columns.  v2 keeps only valid frames/residues: the device
   processes min(nv, 1024) frames (one 128-frame tile per core) x 3*nv
   columns — ~4x less work for nv~1024.  Overflow frames (nv mod 1024,
   when small) are summed exactly on the host in numpy (O(nv) of the
   O(nv^2) total).
2. SCALAR-FIRST POST-PROCESSING.  v1: DVE clamp from PSUM (1x fp32,
   1.19us/KFD) -> ACT sqrt+accum (1.24us/KFD) — two ~1x passes.  v2: ACT
   reads PSUM directly, sqrt -> SBUF bf16 (the one unavoidable 1x pass,
   (172+FD)/1.2ns), then DVE does min(dist,10) + free-axis sum in ONE
   tensor_scalar with accum_out — bf16/SBUF/step-1 keeps all fast perf
   modes (4x_2P: (58+FD/4)/0.96ns).  Steady state is ACT-paced.
3. sqrt(d2 + b), b=4e-3, guards bf16-split cancellation (d2_split can be
   ~-1e-3 where true d2 ~ 0; sqrt(neg) = NaN).  Systematic effect on the
   loss ~ +b/(2*dist) per element ~ 5e-5 relative — far under the 2e-2
   gate (validated vs reference).

Schedule: input [A (tpc*128) | B (3*nv)] per core arrives as ~1-group
DMA pieces on the SP HW-DGE ring so group-g matmuls wait only on piece g;
the ACT table load + bias-const waits are absorbed at t=0 by a dummy
activation chain (bias tile memset by the idle DVE); a standalone
LDWEIGHTS absorbs piece-0's wait on the PE.  PSUM: one (128,1024) fp32
tile per column group, <=8 banks, no reuse for the common nv<=1024 case.

Toolchain constraint: this walrus build allows ONE semaphore wait per
instruction; remaining multi-wait instructions (the Tile exit drain) are
split onto single-wait no-ops by _split_multi_waits.  Tile's entry/exit
all-engine barriers run in sem-only form (no per-engine drains).
"""

import sys

import numpy as np

for _p in ("/opt/trn_rl_repo",):
    if _p not in sys.path:
        sys.path.insert(0, _p)

import ml_dtypes
import concourse.bass as bass
import concourse.tile as tile
from concourse import mybir
from concourse.bass_utils import run_bass_kernel_spmd

L = 2048
N_CORES = 8
K = 28         # 7*8/2 upper-triangle pairs
KS = 3 * K     # 84: three bf16 cross terms stacked on the contraction axis
CHUNK = 512    # matmul N (one PSUM bank)
GROUP = 1024   # cols per ACT/DVE instruction (2 chunks, one 2-bank PSUM tile)
CLAMP = 10.0
B_EPS = 4e-3   # bias under the sqrt; guards split-cancellation negatives

_PAIRS = [(a, b) for a in range(7) for b in range(a, 7)]
_DIAG_COLS = [k for k, (a, b) in enumerate(_PAIRS) if a == b]


def _split(x):
    hi = x.astype(ml_dtypes.bfloat16)
    lo = (x - hi.astype(np.float32)).astype(ml_dtypes.bfloat16)
    return hi, lo


def _host_prep(pred_coords, true_coords, pred_rotation, pred_translation,
               true_rotation, true_translation, idx):
    """Quadratic-form factors for the compacted (valid-only) problem.

    Returns (B (84, 3*nv) bf16, A (nv, 84) bf16, Qv (nv, 28) f64,
    P64 (28, 3*nv) f64).  All O(L) flops."""
    pc = np.asarray(pred_coords, np.float64)[idx]
    tc = np.asarray(true_coords, np.float64)[idx]
    pR = np.asarray(pred_rotation, np.float64)[idx]
    pT = np.asarray(pred_translation, np.float64)[idx]
    tR = np.asarray(true_rotation, np.float64)[idx]
    tT = np.asarray(true_translation, np.float64)[idx]
    nv = len(idx)

    UT = np.concatenate([
        pc.reshape(nv * 3, 3).T,
        tc.reshape(nv * 3, 3).T,
        np.ones((1, nv * 3)),
    ], axis=0)  # (7, 3*nv)

    offp = np.einsum('ic,idc->id', pT, pR)
    offt = np.einsum('ic,idc->id', tT, tR)
    W = np.concatenate([pR, -tR, -(offp - offt)[:, :, None]], axis=2)  # (nv, 3, 7)
    Q = np.einsum('ida,idb->iab', W, W)  # (nv, 7, 7)

    Qv = np.stack([Q[:, a, b] * (1.0 if a == b else 2.0) for (a, b) in _PAIRS],
                  axis=1)  # (nv, 28) f64
    P64 = np.stack([UT[a] * UT[b] for (a, b) in _PAIRS], axis=0)  # (28, 3*nv)

    Ph, Pl = _split(P64.astype(np.float32))
    Qh, Ql = _split(Qv.astype(np.float32))
    B = np.concatenate([Ph, Pl, Ph], axis=0)   # (84, 3*nv)
    A = np.concatenate([Qh, Qh, Ql], axis=1)   # (nv, 84)
    return np.ascontiguousarray(B), np.ascontiguousarray(A), Qv, P64


def _split_multi_waits(nc):
    """The TPB instruction encodings used by this walrus build carry a single
    semaphore wait.  Tile can emit several waits on one instruction (notably
    the kernel-tail drain).  Split the extras onto same-engine no-ops placed
    immediately before the instruction — engine-order execution makes this
    semantically identical."""
    for bbw in nc.main_func.blocks:
        il = bbw.instructions
        out = []
        changed = False
        for ins in il:
            si = ins.sync_info
            if si is not None and len(si.on_wait) > 1:
                waits = list(si.on_wait)
                for idx_, w in enumerate(waits[:-1]):
                    out.append(mybir.InstNoOp(
                        name=f"{ins.name}-waitsplit{idx_}",
                        engine=ins.engine,
                        sync_info=mybir.SyncInfo(on_wait=[w], on_update=[]),
                    ))
                si.on_wait = [waits[-1]]
                changed = True
            out.append(ins)
        if changed:
            bbw.instructions = out


def _build_program(tpc, widths, split_waits=True):
    """tpc: frame tiles per core (usually 1); widths: per-group column
    widths (each <= GROUP, even)."""
    _orig_aeb = bass.Bass.all_engine_barrier
    bass.Bass.all_engine_barrier = (
        lambda self, *, sem_only=False: _orig_aeb(self, sem_only=True))
    try:
        nc = _build_program_inner(tpc, widths)
    finally:
        bass.Bass.all_engine_barrier = _orig_aeb
    if split_waits:
        _split_multi_waits(nc)
    return nc


def _build_program_inner(tpc, widths):
    f32 = mybir.dt.float32
    bf16 = mybir.dt.bfloat16
    C = sum(widths)
    A_COLS = tpc * 128
    NGROUPS = tpc * len(widths)
    nW = len(widths)

    nc = bass.Bass()
    inp = nc.declare_dram_parameter("inp", [KS, A_COLS + C], bf16,
                                    isOutput=False)
    fsums = nc.declare_dram_parameter("fsums", [128, NGROUPS], f32,
                                      isOutput=True)

    # Input DMA pieces: piece 0 (SP ring) = all A tiles + first two chunks;
    # piece 1 (SP) and piece 2.. (GPSIMD/SWDGE ring, idle otherwise) cover
    # the rest two chunks at a time, so transfers overlap across rings and
    # chunk-g matmuls wait on exactly one piece's sem.
    bounds = [0]
    off = A_COLS
    for gi in range(0, nW, 2):
        off += sum(widths[gi:gi + 2])
        bounds.append(off)
    bounds[-1] = A_COLS + C
    piece_engines = ["sync", "sync"] + ["gpsimd"] * max(0, len(bounds) - 3)

    # acc is split into two tiles so the first output DMA (issued as soon as
    # the first tile's columns are complete) doesn't false-depend on later
    # DVE writes (Tile tracks deps per-tile).
    n_acc_a = min(4, NGROUPS)

    with tile.TileContext(nc) as tc:
        with tc.tile_pool(name="const", bufs=1) as const_pool, \
             tc.tile_pool(name="dist", bufs=2 * NGROUPS) as dist_pool, \
             tc.tile_pool(name="ps", bufs=min(NGROUPS, 7), space="PSUM") as ps:
            data = const_pool.tile([KS, A_COLS + C], bf16)
            for i in range(len(bounds) - 1):
                eng = getattr(nc, piece_engines[i])
                eng.dma_start(data[:, bounds[i]:bounds[i + 1]],
                              inp[:, bounds[i]:bounds[i + 1]])

            acc_a = const_pool.tile([128, n_acc_a], f32)
            if NGROUPS > n_acc_a:
                acc_b = const_pool.tile([128, NGROUPS - n_acc_a], f32)
            else:
                acc_b = None
            bias_t = const_pool.tile([128, 1], f32)
            scratch_t = const_pool.tile([128, 1], f32)
            # Idle DVE fills the sqrt-bias const; the dummy activation chain
            # on ACT absorbs the bias wait AND triggers the sqrt table load
            # (PWP, ~1.3us) at t~0, hidden under the input DMA.
            nc.vector.memset(bias_t[:], B_EPS)
            nc.scalar.activation(scratch_t[:], bias_t[:],
                                 mybir.ActivationFunctionType.Sqrt,
                                 bias=bias_t[:, 0:1])
            nc.scalar.activation(scratch_t[:], bias_t[:],
                                 mybir.ActivationFunctionType.Sqrt,
                                 bias=bias_t[:, 0:1])

            for t in range(tpc):
                col = A_COLS
                for gi, w in enumerate(widths):
                    g = t * nW + gi
                    if gi == 0:
                        # Standalone LDWEIGHTS absorbs the DMA-queue wait on
                        # the PE so the real matmuls only wait on their PSUM
                        # slot (single-wait-per-instruction limit).
                        nc.tensor.ldweights(data[:, t * 128:t * 128 + 128])
                    d2 = ps.tile([128, w], f32, tag="d2")
                    nc.tensor.matmul(
                        d2[:],
                        data[:, t * 128:(t + 1) * 128],
                        data[:, col:col + w],
                        start=True, stop=True,
                    )
                    dist = dist_pool.tile([128, w], bf16, tag="dist")
                    nc.scalar.activation(
                        dist[:], d2[:],
                        mybir.ActivationFunctionType.Sqrt,
                        bias=bias_t[:, 0:1],
                    )
                    clamp = dist_pool.tile([128, w], bf16, tag="clamp")
                    # With accum_out, op1 is the reduction op (walrus
                    # TensorScalarPtrReduce form): out = min(in0, 10),
                    # accum_out = sum(out).
                    if g < n_acc_a:
                        acc_ap = acc_a[:, g:g + 1]
                    else:
                        acc_ap = acc_b[:, g - n_acc_a:g - n_acc_a + 1]
                    nc.vector.tensor_scalar(
                        out=clamp[:], in0=dist[:],
                        scalar1=CLAMP, scalar2=None,
                        op0=mybir.AluOpType.min,
                        op1=mybir.AluOpType.add,
                        accum_out=acc_ap,
                    )
                    col += w

            nc.sync.dma_start(fsums[:, 0:n_acc_a], acc_a[:])
            if acc_b is not None:
                nc.sync.dma_start(fsums[:, n_acc_a:], acc_b[:])
    return nc


_PROGRAM_CACHE = {}


def _get_program(tpc, widths):
    key = (tpc, tuple(widths))
    if key not in _PROGRAM_CACHE:
        _PROGRAM_CACHE[key] = _build_program(tpc, widths)
    return _PROGRAM_CACHE[key]


def kernel(pred_coords, true_coords, pred_rotation, pred_translation,
           true_rotation, true_translation, mask, **_run_kwargs):
    mask = np.asarray(mask)
    idx = np.nonzero(mask != 0)[0]
    nv = len(idx)
    if nv == 0:
        out = np.float32(0.0)
        return (out, None) if _run_kwargs else out

    B, A, Qv, P64 = _host_prep(pred_coords, true_coords, pred_rotation,
                               pred_translation, true_rotation,
                               true_translation, idx)
    C = 3 * nv

    # Frame split: device takes n_dev = tpc*1024 (one or more full 128-frame
    # tiles per core); a small overflow is summed exactly on the host.
    tpc = max(1, int(round(nv / 1024)))
    n_dev = min(nv, tpc * 1024)
    n_off = nv - n_dev

    # Column chunks: full CHUNKs (one matmul / ACT / DVE instruction each)
    # then an (even) remainder.
    C_pad = C + (C & 1)
    widths = [CHUNK] * (C_pad // CHUNK)
    if C_pad % CHUNK:
        widths.append(C_pad % CHUNK)
    NGROUPS = tpc * len(widths)

    A_pad = np.zeros((tpc * 1024, KS), A.dtype)
    A_pad[:n_dev] = A[:n_dev]
    B_pad = np.zeros((KS, C_pad), B.dtype)
    B_pad[:, :C] = B

    in_maps = []
    for c in range(N_CORES):
        a_c = A_pad[c * tpc * 128:(c + 1) * tpc * 128].T  # (84, tpc*128)
        in_maps.append({"inp": np.ascontiguousarray(
            np.concatenate([a_c, B_pad], axis=1))})

    nc = _get_program(tpc, widths)
    res = run_bass_kernel_spmd(nc, in_maps, list(range(N_CORES)),
                               **_run_kwargs)

    numer = 0.0
    sqrt_b = float(np.sqrt(B_EPS))  # pad-column contribution (approx; tiny)
    n_pad_cols = C_pad - C
    for c in range(N_CORES):
        fs = np.asarray(res.results[c]["fsums"], np.float64)  # (128, NGROUPS)
        for t in range(tpc):
            f0 = (c * tpc + t) * 128
            cnt = min(128, max(0, n_dev - f0))
            if cnt == 0:
                continue
            s = fs[:cnt, t * len(widths):(t + 1) * len(widths)].sum()
            s -= cnt * n_pad_cols * sqrt_b
            numer += s

    if n_off:
        # Exact host sum for the overflow frames (O(n_off * 3nv) elements).
        d2 = Qv[n_dev:nv] @ P64  # (n_off, 3*nv)
        numer += np.minimum(np.sqrt(np.maximum(d2, 0.0)), CLAMP).sum()

    denom = float(nv) ** 2 * 3.0 + 1e-8
    out = np.float32(numer / denom / 10.0)
    if _run_kwargs:
        return out, res
    return out
